# revision 14
# baseline (speedup 1.0000x reference)
"""Causal self-attention (B=2, T=2048, D=1024, H=16, Dh=64) on 8 NeuronCores.

Sharding: tensor-parallel over heads. Core c owns heads {2c, 2c+1}:
  - QKV: computes q/k/v columns c*128:(c+1)*128 of each section.
      q,k are produced transposed (qT/kT: [128 qkv-cols, tokens]) via
      out = w3_slice.T @ x.T matmuls; v is produced in natural layout
      ([tokens, 128 v-cols]) via PE transposes of the vT chunks.
  - Attention: for each (batch, q-chunk of 512 queries, k-tile of 128):
      S^T = K_h @ Q_h.T from kT/qT (both heads packed in the 128x128 PE
      array via disjoint 64-row groups -> the two head matmuls run
      concurrently), exp on ACT per k-tile ([128, 1024] covering both
      heads; no max subtraction needed: |S*scale| <= ~6), causal mask via
      affine_select on diagonal k-tiles (fill=0 after exp), then out^T
      accumulated as V'.T @ P^T where V' = [V | ones]: row 64 of the PSUM
      accumulator is the softmax denominator.
  - Projection: partial out^T = w_proj_slice.T applied per 128-row slice;
      per-core partial [1024, 4096] outputs are summed on the host.

Pipeline notes (what makes this fast):
  - PSUM budget (8 banks): S pool 2 bufs x [128,1024] (4 banks, so the
    S matmuls of k-tile t+2 overlap the exp of k-tile t), PV accumulators
    2 x [65,512] (2 banks), qkv/proj/transpose pool 2 x [128,512]
    (2 banks). Double-buffered S keeps the PE fed during exp, which keeps
    the HAM clock gate at 8/8 (2.4 GHz) instead of cold 1.2 GHz.
  - Emission order qkv(0), attn(0), qkv(1), attn(1): batch 1's QKV
    matmuls fill the PE during attn(0)'s ACT-bound stretches.
  - Softmax denominators of both heads are batched into one [2,512]
    reciprocal and one DRAM bounce (partition-broadcast read on the
    GPSIMD SWDGE queue), off the critical path.
  - PSUM drains use nc.any so the Tile scheduler balances ACT/DVE.

All matmuls run in float32r (4-byte data, reduced-precision multiply,
1 cycle/row for moving dims >= 256 -- 4x faster than plain fp32).
"""

import numpy as np

D_MODEL = 1024
B, T = 2, 2048
RC = 128  # per-core qkv columns per q/k/v section == per-core w_proj rows
M = B * T
N_CORES = 8

_prog_cache = {}
_last_results = None  # BassKernelResults of the most recent run (for profiling)


def build_program(Tb=T, use_vbias=False):
    from contextlib import ExitStack

    import concourse.bass as bass
    import concourse.tile as tile
    from concourse import bacc, mybir
    from concourse.tile import add_dep_helper

    f32 = mybir.dt.float32
    f32r = mybir.dt.float32r
    EXP = mybir.ActivationFunctionType.Exp
    MULT = mybir.AluOpType.mult
    IS_GE = mybir.AluOpType.is_ge

    mc_per_b = Tb // 512  # x/m chunks of 512 tokens per batch
    mt_per_b = Tb // 128  # v tiles of 128 tokens per batch
    n_qc = Tb // 512      # query chunks per batch

    nc = bacc.Bacc("TRN2", target_bir_lowering=False, debug=False)
    xq = nc.dram_tensor("xq", [B * Tb // 512, 128, 8, 512], f32r,
                        kind="ExternalInput").ap()
    w3 = nc.dram_tensor("w3", [D_MODEL, 3 * RC], f32r, kind="ExternalInput").ap()
    wp = nc.dram_tensor("wp", [RC, D_MODEL], f32r, kind="ExternalInput").ap()
    bqk = nc.dram_tensor("bqk", [RC, 2], f32, kind="ExternalInput").ap()
    ident = nc.dram_tensor("ident", [128, 128], f32r, kind="ExternalInput").ap()
    bv = None
    if use_vbias:
        bv = nc.dram_tensor("bv", [RC, 1], f32, kind="ExternalInput").ap()
    out_d = nc.dram_tensor("out", [D_MODEL, B * Tb], f32, kind="ExternalOutput").ap()
    scr_d = nc.dram_tensor("scr", [B * n_qc, 2, 512], f32).ap()  # denom bounce
    scr2_d = nc.dram_tensor("scr2", [B * n_qc, 64, 16], f32).ap()  # recip bounce

    w3_r = w3.rearrange("(kt p) n -> p kt n", p=128)     # [128, 8, 384]
    out_r = out_d.rearrange("(nt p) m -> p nt m", p=128)  # [128, 8, Mb]

    with tile.TileContext(nc) as tc:
        with ExitStack() as ctx:
            singles = ctx.enter_context(tc.tile_pool(name="singles", bufs=1))
            xpool = ctx.enter_context(tc.tile_pool(name="xpool", bufs=3))
            ptp = ctx.enter_context(tc.tile_pool(name="ptp", bufs=3))
            pt2p = ctx.enter_context(tc.tile_pool(name="pt2p", bufs=2))
            rbp = ctx.enter_context(tc.tile_pool(name="rbp", bufs=2))
            vtp = ctx.enter_context(tc.tile_pool(name="vtp", bufs=2))
            pvcp = ctx.enter_context(tc.tile_pool(name="pvcp", bufs=3))
            obp = ctx.enter_context(tc.tile_pool(name="obp", bufs=4))
            ps_a = ctx.enter_context(tc.tile_pool(name="ps_a", bufs=2, space="PSUM"))
            ps_s = ctx.enter_context(tc.tile_pool(name="ps_s", bufs=2, space="PSUM"))
            ps_pv = ctx.enter_context(tc.tile_pool(name="ps_pv", bufs=2, space="PSUM"))

            # identity first (tiny), then PE warmup matmuls so the HAM clock
            # gate is released by the time the first x chunk lands
            id_sb = singles.tile([128, 128], f32r, tag="ident")
            nc.sync.dma_start(id_sb, ident)
            wu_ps = ps_a.tile([128, 512], f32, tag="mm")
            for _ in range(20):
                nc.tensor.matmul(wu_ps[:, 0:128], id_sb, id_sb,
                                 start=True, stop=True)

            # x chunks prefetched on the SP HWDGE ring; weights go through
            # the ACT HWDGE ring so the two streams don't serialize
            x_tiles, x_dmas = [], []
            for mc in range(B * mc_per_b):
                x_sb = xpool.tile([128, 8, 512], f32r, tag="x")
                xd = nc.sync.dma_start(x_sb, xq[mc])
                if mc == 1:
                    # chunk 0 gets the full HBM bandwidth: QKV can't start
                    # without it, so its arrival time is the kernel prologue
                    add_dep_helper(xd.ins, x_dmas[0].ins,
                                   reason="x DMA pacing")
                elif mc >= 2:
                    add_dep_helper(xd.ins, x_dmas[mc - 2].ins,
                                   reason="x DMA pacing")
                x_tiles.append(x_sb)
                x_dmas.append(xd)

            w3_sb = singles.tile([128, 8, 3 * RC], f32r, tag="w3")
            nc.scalar.dma_start(w3_sb, w3_r)
            wp_sb = singles.tile([128, D_MODEL], f32r, tag="wp")
            nc.scalar.dma_start(wp_sb, wp)
            bqk_sb = singles.tile([RC, 2], f32, tag="bqk")
            nc.scalar.dma_start(bqk_sb, bqk)
            bv_sb = None
            if use_vbias:
                bv_sb = singles.tile([RC, 1], f32, tag="bv")
                nc.scalar.dma_start(bv_sb, bv)

            qT, kT, vb, aT = {}, {}, {}, {}
            for b in range(B):
                qT[b] = singles.tile([128, Tb], f32r, tag=f"qT{b}", name=f"qT{b}")
                kT[b] = singles.tile([128, Tb], f32r, tag=f"kT{b}", name=f"kT{b}")
                vb[b] = singles.tile([128, mt_per_b, 130], f32r, tag=f"vb{b}",
                                     name=f"vb{b}")
                aT[b] = singles.tile([128, Tb], f32r, tag=f"aT{b}", name=f"aT{b}")
                # ones columns for the softmax-denominator rows of PV
                nc.vector.memset(vb[b][:, :, 64:65].bitcast(f32), 1.0)
                nc.vector.memset(vb[b][:, :, 129:130].bitcast(f32), 1.0)

            def emit_qkv(b):
                for mci in range(mc_per_b):
                    mc = b * mc_per_b + mci
                    x_sb = x_tiles[mc]
                    # qT / kT / vT: out[qkvcol, m] accumulated over 8 k-tiles
                    vTs = None
                    for nt in range(3):
                        ps = ps_a.tile([128, 512], f32, tag="mm")
                        for kt in range(8):
                            nc.tensor.matmul(
                                ps,
                                w3_sb[:, kt, nt * RC:(nt + 1) * RC],
                                x_sb[:, kt, :],
                                start=(kt == 0), stop=(kt == 7),
                            )
                        if nt < 2:
                            dest = qT[b] if nt == 0 else kT[b]
                            nc.vector.tensor_scalar_add(
                                dest[:, mci * 512:(mci + 1) * 512], ps,
                                bqk_sb[:, nt:nt + 1],
                            )
                        else:
                            vTs = vtp.tile([128, 512], f32r, tag="vT")
                            nc.vector.tensor_copy(vTs, ps)
                    # transpose vT chunks into natural [tokens, vcol] layout
                    tp = ps_a.tile([128, 512], f32, tag="mm")
                    for ms in range(4):
                        nc.tensor.transpose(
                            tp[:, ms * 128:(ms + 1) * 128].bitcast(f32r),
                            vTs[:, ms * 128:(ms + 1) * 128],
                            id_sb,
                        )
                    for ms in range(4):
                        mt = mci * 4 + ms
                        sl = tp[:, ms * 128:(ms + 1) * 128].bitcast(f32r)
                        nc.vector.tensor_copy(vb[b][:, mt, 0:64], sl[:, 0:64])
                        nc.vector.tensor_copy(vb[b][:, mt, 65:129], sl[:, 64:128])

            def emit_proj(b, qc):
                for np_ in range(4):
                    ob = obp.tile([128, 2, 512], f32, tag="ob")
                    for j in range(2):
                        nt = np_ * 2 + j
                        ps = ps_a.tile([128, 512], f32, tag="mm")
                        nc.tensor.matmul(
                            ps,
                            wp_sb[:, nt * 128:(nt + 1) * 128],
                            aT[b][:, qc * 512:(qc + 1) * 512],
                            start=True, stop=True,
                        )
                        nc.vector.tensor_copy(ob[:, j, :], ps)
                    nc.sync.dma_start(
                        out_r[:, np_ * 2:np_ * 2 + 2,
                              b * Tb + qc * 512: b * Tb + (qc + 1) * 512],
                        ob,
                    )

            pending_proj = []

            def emit_attn(b):
                for qc in range(n_qc):
                    nkt = (qc + 1) * 4
                    pvs = (
                        ps_pv.tile([65, 512], f32, tag="pv", name="pv0"),
                        ps_pv.tile([65, 512], f32, tag="pv", name="pv1"),
                    )
                    # one k-tile per S psum tile; ps_s is double-buffered so
                    # the S matmuls of k-tile t+2 run while exp(t) drains.
                    # The two heads' S matmuls use disjoint 64-row groups of
                    # the PE array and run concurrently.
                    for kt in range(nkt):
                        s = ps_s.tile([128, 2, 512], f32, tag="s")
                        for h in (0, 1):
                            nc.tensor.matmul(
                                s[:, h, :],
                                kT[b][h * 64:(h + 1) * 64,
                                      kt * 128:(kt + 1) * 128],
                                qT[b][h * 64:(h + 1) * 64,
                                      qc * 512:(qc + 1) * 512],
                                start=True, stop=True,
                            )
                        pt = ptp.tile([128, 2, 512], f32r, tag="pt")
                        nc.scalar.activation(pt, s, EXP, scale=0.125)
                        if kt >= nkt - 4:  # diagonal k-tiles: causal mask
                            pt2 = pt2p.tile([128, 2, 512], f32r, tag="pt2")
                            for h in (0, 1):
                                nc.gpsimd.affine_select(
                                    pt2[:, h, :],
                                    pt[:, h, :],
                                    pattern=[[1, 512]],
                                    compare_op=IS_GE,
                                    fill=0.0,
                                    base=qc * 512 - kt * 128,
                                    channel_multiplier=-1,
                                )
                            psrc = pt2
                        else:
                            psrc = pt
                        for h in (0, 1):
                            nc.tensor.matmul(
                                pvs[h],
                                vb[b][:, kt, h * 65:(h + 1) * 65],
                                psrc[:, h, :],
                                start=(kt == 0), stop=(kt == nkt - 1),
                            )
                    # Normalize: the denominator rows (64) of the pvc copies
                    # are bounced through DRAM and read back spread over 64
                    # partitions, so the reciprocal runs on 64 DVE lanes
                    # (a [1,512] single-lane reciprocal measures 3.3us).
                    # A second bounce broadcasts the reciprocals across
                    # partitions; the multiply runs on GPSIMD, which shares
                    # the bounce dependency and so blocks nothing else.
                    # Small scratch DMAs ride the GPSIMD SWDGE queue to keep
                    # the sync ring (x chain + out stores) and the ACT queue
                    # (exp stream) free of slow-dependency waits.
                    slot = b * n_qc + qc
                    pvcs, d1s = [], []
                    tail = (b == B - 1) and (qc == n_qc - 1)
                    for h in (0, 1):
                        pvc = pvcp.tile([96, 512], f32, tag="pvc")
                        nc.vector.tensor_copy(pvc[0:65, :], pvs[h])
                        pvcs.append(pvc)
                        if not tail:
                            d1s.append(
                                nc.gpsimd.dma_start(scr_d[slot, h:h + 1, :],
                                                    pvc[64:65, :])
                            )
                    if not tail:
                        a0 = scr_d[slot]
                        rs_t = rbp.tile([64, 16], f32, tag="rs")
                        d2 = nc.gpsimd.dma_start(rs_t, bass.AP(
                            tensor=a0.tensor, offset=a0.offset,
                            ap=[[16, 64], [1, 16]]))
                        add_dep_helper(d2.ins, d1s[0].ins,
                                       reason="scr bounce RAW")
                        add_dep_helper(d2.ins, d1s[1].ins,
                                       reason="scr bounce RAW")
                        rc_t = rbp.tile([64, 16], f32, tag="rc")
                        nc.vector.reciprocal(rc_t, rs_t)
                        a1 = scr2_d[slot]
                        d3 = nc.gpsimd.dma_start(a1, rc_t)
                        rb_t = rbp.tile([64, 2, 512], f32, tag="rb")
                        d4 = nc.gpsimd.dma_start(rb_t, bass.AP(
                            tensor=a1.tensor, offset=a1.offset,
                            ap=[[0, 64], [512, 2], [1, 512]]))
                        add_dep_helper(d4.ins, d3.ins, reason="scr2 bounce RAW")
                        for h in (0, 1):
                            dst = aT[b][h * 64:(h + 1) * 64,
                                        qc * 512:(qc + 1) * 512]
                            nc.gpsimd.tensor_tensor(
                                dst, pvcs[h][0:64, :], rb_t[:, h, :],
                                op=MULT,
                            )
                            if use_vbias:
                                nc.vector.tensor_scalar_add(
                                    dst, dst, bv_sb[h * 64:(h + 1) * 64, 0:1]
                                )
                    else:
                        # Tail chunk: the DRAM bounce latency (~13us) would be
                        # fully exposed, so normalize on-chip instead. DVE
                        # 32x32 block-transpose scatters the denominator row
                        # over 32 partitions (denoms land in columns 0 mod 32),
                        # a strided reciprocal runs on 32 lanes, a second
                        # block-transpose packs the reciprocals back into a
                        # single row, and a K=1 matmul broadcasts that row
                        # across 64 psum partitions for the multiply.
                        for h in (0, 1):
                            scat = rbp.tile([32, 512], f32, tag="sc")
                            nc.vector.transpose(scat, pvcs[h][64:96, :])
                            rin = rbp.tile([32, 512], f32, tag="ri")
                            sc_v = scat.rearrange("p (bk i) -> p bk i", i=32)
                            ri_v = rin.rearrange("p (bk i) -> p bk i", i=32)
                            nc.vector.reciprocal(ri_v[:, :, 0:1], sc_v[:, :, 0:1])
                            row = rbp.tile([32, 512], f32, tag="ro")
                            nc.vector.transpose(row, rin)
                            rb_h = rbp.tile([64, 512], f32, tag="rbh")
                            nc.gpsimd.partition_broadcast(rb_h, row[0:1, :])
                            dst = aT[b][h * 64:(h + 1) * 64,
                                        qc * 512:(qc + 1) * 512]
                            nc.vector.tensor_tensor(
                                dst, pvcs[h][0:64, :], rb_h,
                                op=MULT,
                            )
                            if use_vbias:
                                nc.vector.tensor_scalar_add(
                                    dst, dst, bv_sb[h * 64:(h + 1) * 64, 0:1]
                                )
                    # projection runs one q-chunk late: by the time the PE
                    # drains this chunk's matmuls, the previous chunk's
                    # normalize chain has finished, so the proj matmuls never
                    # stall the in-order PE queue.
                    while pending_proj:
                        emit_proj(*pending_proj.pop(0))
                    pending_proj.append((b, qc))

            emit_qkv(0)
            emit_attn(0)
            emit_qkv(1)
            emit_attn(1)
            while pending_proj:
                emit_proj(*pending_proj.pop(0))

    nc.compile()
    return nc


def make_in_maps(x, w_qkv, b_qkv, use_vbias):
    """Host-side shard prep. Returns per-core input maps (w_proj added later)."""
    Mx = x.shape[0] * x.shape[1]
    # [chunks, 128p, 8kt, 512m]: per-partition-contiguous 16 KiB blocks so
    # each chunk DMA uses 128 big descriptors instead of 1024 2-KiB ones
    xq = np.ascontiguousarray(
        x.reshape(Mx // 512, 512, 8, 128).transpose(0, 3, 2, 1)
    )
    in_maps = []
    for c in range(N_CORES):
        w3c = np.ascontiguousarray(
            np.concatenate(
                [w_qkv[:, s * D_MODEL + c * RC: s * D_MODEL + (c + 1) * RC]
                 for s in range(3)],
                axis=1,
            )
        )
        bqkc = np.ascontiguousarray(
            np.stack(
                [b_qkv[c * RC:(c + 1) * RC],
                 b_qkv[D_MODEL + c * RC: D_MODEL + (c + 1) * RC]],
                axis=1,
            )
        )
        im = {"xq": xq, "w3": w3c, "bqk": bqkc,
              "ident": np.eye(128, dtype=np.float32)}
        if use_vbias:
            im["bv"] = np.ascontiguousarray(
                b_qkv[2 * D_MODEL + c * RC: 2 * D_MODEL + (c + 1) * RC][:, None]
            )
        in_maps.append(im)
    return in_maps


def kernel(x, w_qkv, b_qkv, w_proj, b_proj):
    from concourse.bass_utils import run_bass_kernel_spmd

    x = np.asarray(x, dtype=np.float32)
    w_qkv = np.asarray(w_qkv, dtype=np.float32)
    b_qkv = np.asarray(b_qkv, dtype=np.float32)
    w_proj = np.asarray(w_proj, dtype=np.float32)
    b_proj = np.asarray(b_proj, dtype=np.float32)

    use_vbias = bool(np.any(b_qkv[2 * D_MODEL:]))
    key = (T, use_vbias)
    if key not in _prog_cache:
        _prog_cache[key] = build_program(T, use_vbias)
    nc = _prog_cache[key]

    in_maps = make_in_maps(x, w_qkv, b_qkv, use_vbias)
    for c in range(N_CORES):
        in_maps[c]["wp"] = np.ascontiguousarray(w_proj[c * RC:(c + 1) * RC, :])

    res = run_bass_kernel_spmd(nc, in_maps, core_ids=list(range(N_CORES)))
    global _last_results
    _last_results = res
    total = res.results[0]["out"].copy()
    for c in range(1, N_CORES):
        total += res.results[c]["out"]
    out = total.T.reshape(B, T, D_MODEL) + b_proj[None, None, :]
    return np.ascontiguousarray(out.astype(np.float32))
